# revision 15
# baseline (speedup 1.0000x reference)
"""Multi-head attention (b=4, n=2048, dim=768, 12 heads) on 8 TRN2 NeuronCores.

Sharding: core c handles batch c//2 and head-group c%2 (6 heads).  Each core
computes its heads' contribution projected through its slice of Wo, returning
a partial [2048, 768] f32 output; the host sums core pairs and adds the bias.

Per-core kernel (all TensorE-facing data in bf16, accumulation in f32):
  P1: QT/KT = W^T x^T (feature-major), V token-major with a ones column per
      head (V' = [V_h | 1]) so the PV matmul also produces softmax denominators.
  P2: per (head, 512-wide i-block): ST[j,i] = K Q^T via PE, exp on ACT with
      the 1/8 scale folded in (no max subtraction: logits are ~N(0,1), far
      from f32 exp overflow), then OP[65, i] += V'^T exp(ST) accumulated over
      j in PSUM.  Rows 0..63 = unnormalized head output (feature-major),
      row 64 = softmax denominator l[i].
  P3: transpose OP columns to token-major, multiply by 1/l per partition,
      transpose back, and project through Wo with PSUM accumulation.
"""

import os
import sys
import types
import numpy as np
import ml_dtypes

B, N, DIM = 4, 2048, 768
HEADS, DH = 12, 64
HPC = 6                # heads per core
FPC = HPC * DH         # 384 features per core
NCORES = 8
KC = DIM // 128        # 6 contraction chunks
FT = FPC // 128        # 3 feature tiles per core
NT = N // 128          # 16 token chunks of 128
IBS = 512              # i-block size
IB = N // IBS          # 4 i-blocks
BF16 = ml_dtypes.bfloat16

_cache = {}
last_exec_time_ns = None


def _install_ntff_hook():
    try:
        import antenv.axon_hooks  # noqa: F401
        return
    except ImportError:
        pass
    from trn_agent_boot.trn_boot import _ntff_profile_via_ctypes
    hook = _ntff_profile_via_ctypes('/opt/axon/libaxon_pjrt.so')
    mod = types.ModuleType('antenv.axon_hooks')
    mod.get_axon_ntff_profile_hook = lambda: hook
    import antenv
    sys.modules['antenv.axon_hooks'] = mod
    antenv.axon_hooks = mod


def _build_nc():
    from contextlib import ExitStack
    from concourse import bacc
    import concourse.mybir as mybir
    from concourse.tile import TileContext
    from concourse.masks import make_identity

    dt = mybir.dt
    EXP = mybir.ActivationFunctionType.Exp

    nc = bacc.Bacc("TRN2", target_bir_lowering=False, debug=False,
                   num_devices=NCORES)
    xT = nc.dram_tensor("xT", [DIM, N], dt.bfloat16, kind="ExternalInput").ap()
    wq = nc.dram_tensor("wq", [DIM, FPC], dt.bfloat16, kind="ExternalInput").ap()
    wk = nc.dram_tensor("wk", [DIM, FPC], dt.bfloat16, kind="ExternalInput").ap()
    wv = nc.dram_tensor("wv", [DIM, FPC], dt.bfloat16, kind="ExternalInput").ap()
    wo = nc.dram_tensor("wo", [FPC, DIM], dt.bfloat16, kind="ExternalInput").ap()
    out = nc.dram_tensor("out", [N, DIM], dt.float32, kind="ExternalOutput").ap()

    with TileContext(nc) as tc, ExitStack() as ctx:
        const = ctx.enter_context(tc.tile_pool(name="const", bufs=1))
        id_f32 = const.tile([128, 128], dt.float32, tag="idf")
        id_bf = const.tile([128, 128], dt.bfloat16, tag="idb")
        make_identity(nc, id_f32)
        make_identity(nc, id_bf)
        nbias = const.tile([128, 1], dt.float32, tag="nbias")
        nc.vector.memset(nbias[:], -3.2)

        inp = ctx.enter_context(tc.tile_pool(name="inp", bufs=1))
        xts = [inp.tile([128, N], dt.bfloat16, tag=f"xt{k}", name=f"xt{k}")
               for k in range(KC)]
        wqs = [inp.tile([128, FPC], dt.bfloat16, tag=f"wq{k}", name=f"wq{k}")
               for k in range(KC)]
        wks = [inp.tile([128, FPC], dt.bfloat16, tag=f"wk{k}", name=f"wk{k}")
               for k in range(KC)]
        wvs = [inp.tile([128, FPC], dt.bfloat16, tag=f"wv{k}", name=f"wv{k}")
               for k in range(KC)]
        wos = [inp.tile([128, DIM], dt.bfloat16, tag=f"wo{f}", name=f"wo{f}")
               for f in range(FT)]
        for k in range(KC):
            eng = nc.sync if k % 2 == 0 else nc.scalar
            eng.dma_start(out=xts[k][:], in_=xT[k * 128:(k + 1) * 128, :])
        for k in range(KC):
            eng = nc.sync if k % 2 == 1 else nc.scalar
            eng.dma_start(out=wvs[k][:], in_=wv[k * 128:(k + 1) * 128, :])
        for k in range(KC):
            eng = nc.sync if k % 2 == 0 else nc.scalar
            eng.dma_start(out=wks[k][:], in_=wk[k * 128:(k + 1) * 128, :])
            eng.dma_start(out=wqs[k][:], in_=wq[k * 128:(k + 1) * 128, :])
        for f in range(FT):
            nc.scalar.dma_start(out=wos[f][:], in_=wo[f * 128:(f + 1) * 128, :])

        kqv = ctx.enter_context(tc.tile_pool(name="kqv", bufs=1))
        KT = [kqv.tile([128, N], dt.bfloat16, tag=f"kt{f}", name=f"kt{f}")
              for f in range(FT)]
        QT = [kqv.tile([128, N], dt.bfloat16, tag=f"qt{f}", name=f"qt{f}")
              for f in range(FT)]
        VP = [kqv.tile([128, HPC * 128], dt.bfloat16, tag=f"vp{t}", name=f"vp{t}")
              for t in range(NT)]
        opsb = ctx.enter_context(tc.tile_pool(name="opsb", bufs=1))
        OPS = [[opsb.tile([65, IBS], dt.float32, tag=f"op{h}_{ib}",
                          name=f"op{h}_{ib}") for ib in range(IB)]
               for h in range(HPC)]

        # ---- P1: projections ----
        from concourse.bass import broadcast_tensor_aps
        for t in range(NT):
            nc.gpsimd.memset(VP[t][:], 1.0)
        with tc.tile_pool(name="p1ps", bufs=3, space="PSUM") as p1:
            for t in range(NT):
                ps = p1.tile([128, FPC], dt.float32, tag="p1", name=f"vps{t}")
                for k in range(KC):
                    nc.tensor.matmul(ps[:], lhsT=xts[k][:, t * 128:(t + 1) * 128],
                                     rhs=wvs[k][:], start=(k == 0),
                                     stop=(k == KC - 1))
                nc.scalar.copy(
                    VP[t].rearrange("p (h c) -> p h c", c=128)[:, :, 0:64],
                    ps.rearrange("p (h c) -> p h c", c=64))
            for W, DST in ((wks, KT), (wqs, QT)):
                for f in range(FT):
                    for q in range(N // 512):
                        ps = p1.tile([128, 512], dt.float32, tag="p1",
                                     name=f"kqps{f}_{q}")
                        for k in range(KC):
                            nc.tensor.matmul(
                                ps[:], lhsT=W[k][:, f * 128:(f + 1) * 128],
                                rhs=xts[k][:, q * 512:(q + 1) * 512],
                                start=(k == 0), stop=(k == KC - 1))
                        nc.vector.tensor_copy(DST[f][:, q * 512:(q + 1) * 512],
                                              ps[:])

        # ---- P2: attention, exp tiles pack 3 jc-units across head bounds ----
        PACK = 3
        units = [(h, ib, jc) for h in range(HPC) for ib in range(IB)
                 for jc in range(NT)]
        assert len(units) % PACK == 0
        with tc.tile_pool(name="p2st", bufs=2, space="PSUM") as p2st, \
                tc.tile_pool(name="p2op", bufs=2, space="PSUM") as p2op, \
                tc.tile_pool(name="expp", bufs=4) as expp:
            ops = {}
            for g in range(len(units) // PACK):
                pack = units[g * PACK:(g + 1) * PACK]
                st = p2st.tile([128, PACK * IBS], dt.float32, tag="st",
                               name=f"st{g}")
                for u, (h, ib, jc) in enumerate(pack):
                    ktf, qtf, r0 = KT[h // 2], QT[h // 2], (h % 2) * 64
                    nc.tensor.matmul(
                        st[:, u * IBS:(u + 1) * IBS],
                        lhsT=ktf[r0:r0 + 64, jc * 128:(jc + 1) * 128],
                        rhs=qtf[r0:r0 + 64, ib * IBS:(ib + 1) * IBS],
                        start=True, stop=True)
                ex = expp.tile([128, PACK * IBS], dt.bfloat16, tag="ex",
                               name=f"ex{g}")
                nc.scalar.activation(ex[:], st[:], EXP, scale=0.125,
                                     bias=nbias[:])
                for u, (h, ib, jc) in enumerate(pack):
                    if jc == 0:
                        ops[(h, ib)] = p2op.tile([128, IBS], dt.float32,
                                                 tag="op", name=f"opp{h}_{ib}")
                    nc.tensor.matmul(
                        ops[(h, ib)][:], lhsT=VP[jc][:, h * 128:(h + 1) * 128],
                        rhs=ex[:, u * IBS:(u + 1) * IBS],
                        start=(jc == 0), stop=(jc == NT - 1))
                    if jc == NT - 1:
                        op = ops.pop((h, ib))
                        nc.vector.tensor_copy(OPS[h][ib][:], op[0:65, :])

        # ---- P3: normalize + output projection (3-stage pipeline) ----
        with tc.tile_pool(name="p3tr", bufs=2, space="PSUM") as p3tr, \
                tc.tile_pool(name="p3tb", bufs=2, space="PSUM") as p3tb, \
                tc.tile_pool(name="p3pp", bufs=2, space="PSUM") as p3pp, \
                tc.tile_pool(name="otokp", bufs=3) as otokp, \
                tc.tile_pool(name="otnp", bufs=3) as otnp, \
                tc.tile_pool(name="linvp", bufs=3) as linvp, \
                tc.tile_pool(name="outst", bufs=2) as outst:
            otoks = {}
            otns = {}

            def tr_block(isub):
                ib, col = isub // 4, (isub % 4) * 128
                trp = p3tr.tile([128, HPC * 65], dt.float32, tag="tr",
                                name=f"trp{isub}")
                for h in range(HPC):
                    nc.tensor.transpose(trp[:, h * 65:(h + 1) * 65],
                                        OPS[h][ib][0:65, col:col + 128],
                                        id_f32[0:65, 0:65])
                trv = trp.rearrange("p (h c) -> p h c", c=65)
                linv6 = linvp.tile([128, HPC, 1], dt.float32, tag="l6",
                                   name=f"l6{isub}")
                nc.vector.reciprocal(linv6[:], trv[:, :, 64:65])
                otok = otokp.tile([128, FPC], dt.bfloat16, tag="otok",
                                  name=f"otok{isub}")
                a, b = broadcast_tensor_aps(trv[:, :, 0:64], linv6[:])
                nc.vector.tensor_mul(
                    otok.rearrange("p (h c) -> p h c", c=64), a, b)
                otoks[isub] = otok

            def tb_block(isub):
                otok = otoks.pop(isub)
                tbp = p3tb.tile([128, FPC], dt.bfloat16, tag="tb",
                                name=f"tbp{isub}")
                for f in range(FT):
                    nc.tensor.transpose(tbp[:, f * 128:(f + 1) * 128],
                                        otok[:, f * 128:(f + 1) * 128],
                                        id_bf[:])
                otn = otnp.tile([128, FPC], dt.bfloat16, tag="otn",
                                name=f"otn{isub}")
                nc.vector.tensor_copy(otn[:], tbp[:])
                otns[isub] = otn

            def proj_block(isub):
                otn = otns.pop(isub)
                pp = p3pp.tile([128, DIM], dt.float32, tag="pp",
                               name=f"pp{isub}")
                for f in range(FT):
                    nc.tensor.matmul(pp[:, 0:512], lhsT=otn[:, f * 128:(f + 1) * 128],
                                     rhs=wos[f][:, 0:512],
                                     start=(f == 0), stop=(f == FT - 1))
                    nc.tensor.matmul(pp[:, 512:768], lhsT=otn[:, f * 128:(f + 1) * 128],
                                     rhs=wos[f][:, 512:768],
                                     start=(f == 0), stop=(f == FT - 1))
                ob = outst.tile([128, DIM], dt.float32, tag="ob",
                                name=f"ob{isub}")
                nc.vector.tensor_copy(ob[:], pp[:])
                nc.sync.dma_start(out=out[isub * 128:(isub + 1) * 128, :],
                                  in_=ob[:])

            stages = [tr_block, tb_block, proj_block]
            for step in range(NT + 2):
                for depth, fn in enumerate(stages):
                    i = step - depth
                    if 0 <= i < NT:
                        fn(i)

    nc.finalize()
    return nc


def _get_nc():
    if "nc" not in _cache:
        _cache["nc"] = _build_nc()
    return _cache["nc"]


def kernel(x, Wq, Wk, Wv, Wo, bo):
    global last_exec_time_ns
    x = np.asarray(x, dtype=np.float32)
    Wq = np.asarray(Wq, dtype=np.float32)
    Wk = np.asarray(Wk, dtype=np.float32)
    Wv = np.asarray(Wv, dtype=np.float32)
    Wo = np.asarray(Wo, dtype=np.float32)
    bo = np.asarray(bo, dtype=np.float32)

    trace = bool(os.environ.get("BASS_KERNEL_TRACE"))
    if trace:
        _install_ntff_hook()
        import concourse.bass_utils as bass_utils
        bass_utils.upload_artifacts = lambda tmpdir: tmpdir

    nc = _get_nc()
    in_maps = []
    for c in range(NCORES):
        bi, hg = divmod(c, 2)
        s = slice(hg * FPC, (hg + 1) * FPC)
        in_maps.append({
            "xT": np.ascontiguousarray(x[bi].T).astype(BF16),
            "wq": np.ascontiguousarray(Wq[:, s]).astype(BF16),
            "wk": np.ascontiguousarray(Wk[:, s]).astype(BF16),
            "wv": np.ascontiguousarray(Wv[:, s]).astype(BF16),
            "wo": np.ascontiguousarray(Wo[s, :]).astype(BF16),
        })

    from concourse.bass_utils import run_bass_kernel_spmd
    res = run_bass_kernel_spmd(nc, in_maps, list(range(NCORES)), trace=trace)
    last_exec_time_ns = res.exec_time_ns

    parts = [res.results[c]["out"] for c in range(NCORES)]
    full = np.empty((B, N, DIM), np.float32)
    for bi in range(B):
        full[bi] = parts[2 * bi] + parts[2 * bi + 1] + bo[None, :]
    return full


# revision 16
# speedup vs baseline: 1.4259x; 1.4259x over previous
"""Multi-head attention (b=4, n=2048, dim=768, 12 heads) on 8 TRN2 NeuronCores.

Sharding: core c handles batch c//2 and head-group c%2 (6 heads).  Each core
computes its heads' contribution projected through its slice of Wo, returning
a partial [2048, 768] f32 output; the host sums core pairs and adds the bias.

Per-core kernel (all TensorE-facing data in bf16, accumulation in f32):
  P1: QT/KT = W^T x^T (feature-major), V token-major with a ones column per
      head (V' = [V_h | 1]) so the PV matmul also produces softmax denominators.
  P2: per (head, 512-wide i-block): ST[j,i] = K Q^T via PE, exp on ACT with
      the 1/8 scale folded in (no max subtraction: logits are ~N(0,1), far
      from f32 exp overflow), then OP[65, i] += V'^T exp(ST) accumulated over
      j in PSUM.  Rows 0..63 = unnormalized head output (feature-major),
      row 64 = softmax denominator l[i].
  P3: transpose OP columns to token-major, multiply by 1/l per partition,
      transpose back, and project through Wo with PSUM accumulation.
"""

import os
import sys
import types
import numpy as np
import ml_dtypes

B, N, DIM = 4, 2048, 768
HEADS, DH = 12, 64
HPC = 6                # heads per core
FPC = HPC * DH         # 384 features per core
NCORES = 8
KC = DIM // 128        # 6 contraction chunks
FT = FPC // 128        # 3 feature tiles per core
NT = N // 128          # 16 token chunks of 128
IBS = 512              # i-block size
IB = N // IBS          # 4 i-blocks
BF16 = ml_dtypes.bfloat16

_cache = {}
last_exec_time_ns = None


def _install_ntff_hook():
    try:
        import antenv.axon_hooks  # noqa: F401
        return
    except ImportError:
        pass
    from trn_agent_boot.trn_boot import _ntff_profile_via_ctypes
    hook = _ntff_profile_via_ctypes('/opt/axon/libaxon_pjrt.so')
    mod = types.ModuleType('antenv.axon_hooks')
    mod.get_axon_ntff_profile_hook = lambda: hook
    import antenv
    sys.modules['antenv.axon_hooks'] = mod
    antenv.axon_hooks = mod


def _build_nc():
    from contextlib import ExitStack
    from concourse import bacc
    import concourse.mybir as mybir
    from concourse.tile import TileContext
    from concourse.masks import make_identity

    dt = mybir.dt
    EXP = mybir.ActivationFunctionType.Exp

    nc = bacc.Bacc("TRN2", target_bir_lowering=False, debug=False,
                   num_devices=NCORES)
    xT = nc.dram_tensor("xT", [DIM, N], dt.bfloat16, kind="ExternalInput").ap()
    wq = nc.dram_tensor("wq", [DIM, FPC], dt.bfloat16, kind="ExternalInput").ap()
    wk = nc.dram_tensor("wk", [DIM, FPC], dt.bfloat16, kind="ExternalInput").ap()
    wv = nc.dram_tensor("wv", [DIM, FPC], dt.bfloat16, kind="ExternalInput").ap()
    wo = nc.dram_tensor("wo", [FPC, DIM], dt.bfloat16, kind="ExternalInput").ap()
    out = nc.dram_tensor("out", [N, DIM], dt.float32, kind="ExternalOutput").ap()

    with TileContext(nc) as tc, ExitStack() as ctx:
        const = ctx.enter_context(tc.tile_pool(name="const", bufs=1))
        id_f32 = const.tile([128, 128], dt.float32, tag="idf")
        id_bf = const.tile([128, 128], dt.bfloat16, tag="idb")
        make_identity(nc, id_f32)
        make_identity(nc, id_bf)
        nbias = const.tile([128, 1], dt.float32, tag="nbias")
        nc.vector.memset(nbias[:], -3.2)

        inp = ctx.enter_context(tc.tile_pool(name="inp", bufs=1))
        xts = [inp.tile([128, N], dt.bfloat16, tag=f"xt{k}", name=f"xt{k}")
               for k in range(KC)]
        wqs = [inp.tile([128, FPC], dt.bfloat16, tag=f"wq{k}", name=f"wq{k}")
               for k in range(KC)]
        wks = [inp.tile([128, FPC], dt.bfloat16, tag=f"wk{k}", name=f"wk{k}")
               for k in range(KC)]
        wvs = [inp.tile([128, FPC], dt.bfloat16, tag=f"wv{k}", name=f"wv{k}")
               for k in range(KC)]
        wos = [inp.tile([128, DIM], dt.bfloat16, tag=f"wo{f}", name=f"wo{f}")
               for f in range(FT)]
        for k in range(KC):
            nc.sync.dma_start(out=xts[k][:], in_=xT[k * 128:(k + 1) * 128, :])
            nc.sync.dma_start(out=wvs[k][:], in_=wv[k * 128:(k + 1) * 128, :])
        for k in range(KC):
            nc.scalar.dma_start(out=wks[k][:], in_=wk[k * 128:(k + 1) * 128, :])
            nc.scalar.dma_start(out=wqs[k][:], in_=wq[k * 128:(k + 1) * 128, :])
        for f in range(FT):
            nc.scalar.dma_start(out=wos[f][:], in_=wo[f * 128:(f + 1) * 128, :])

        kqv = ctx.enter_context(tc.tile_pool(name="kqv", bufs=1))
        KT = [kqv.tile([128, N], dt.bfloat16, tag=f"kt{f}", name=f"kt{f}")
              for f in range(FT)]
        QT = [kqv.tile([128, N], dt.bfloat16, tag=f"qt{f}", name=f"qt{f}")
              for f in range(FT)]
        VP = [kqv.tile([128, HPC * 128], dt.bfloat16, tag=f"vp{t}", name=f"vp{t}")
              for t in range(NT)]
        opsb = ctx.enter_context(tc.tile_pool(name="opsb", bufs=1))
        OPS = [[opsb.tile([65, IBS], dt.float32, tag=f"op{h}_{ib}",
                          name=f"op{h}_{ib}") for ib in range(IB)]
               for h in range(HPC)]

        # ---- P1: projections ----
        from concourse.bass import broadcast_tensor_aps
        for t in range(NT):
            nc.gpsimd.memset(VP[t][:], 1.0)
        with tc.tile_pool(name="p1ps", bufs=3, space="PSUM") as p1:
            for t in range(NT):
                ps = p1.tile([128, FPC], dt.float32, tag="p1", name=f"vps{t}")
                for k in range(KC):
                    nc.tensor.matmul(ps[:], lhsT=xts[k][:, t * 128:(t + 1) * 128],
                                     rhs=wvs[k][:], start=(k == 0),
                                     stop=(k == KC - 1))
                nc.vector.tensor_copy(
                    VP[t].rearrange("p (h c) -> p h c", c=128)[:, :, 0:64],
                    ps.rearrange("p (h c) -> p h c", c=64))
            for W, DST in ((wks, KT), (wqs, QT)):
                for f in range(FT):
                    for q in range(N // 512):
                        ps = p1.tile([128, 512], dt.float32, tag="p1",
                                     name=f"kqps{f}_{q}")
                        for k in range(KC):
                            nc.tensor.matmul(
                                ps[:], lhsT=W[k][:, f * 128:(f + 1) * 128],
                                rhs=xts[k][:, q * 512:(q + 1) * 512],
                                start=(k == 0), stop=(k == KC - 1))
                        nc.vector.tensor_copy(DST[f][:, q * 512:(q + 1) * 512],
                                              ps[:])

        # ---- P2: attention, exp tiles pack 3 jc-units across head bounds ----
        PACK = 3
        units = [(h, ib, jc) for h in range(HPC) for ib in range(IB)
                 for jc in range(NT)]
        assert len(units) % PACK == 0
        with tc.tile_pool(name="p2st", bufs=2, space="PSUM") as p2st, \
                tc.tile_pool(name="p2op", bufs=2, space="PSUM") as p2op, \
                tc.tile_pool(name="expp", bufs=4) as expp:
            ops = {}
            for g in range(len(units) // PACK):
                pack = units[g * PACK:(g + 1) * PACK]
                st = p2st.tile([128, PACK * IBS], dt.float32, tag="st",
                               name=f"st{g}")
                for u, (h, ib, jc) in enumerate(pack):
                    ktf, qtf, r0 = KT[h // 2], QT[h // 2], (h % 2) * 64
                    nc.tensor.matmul(
                        st[:, u * IBS:(u + 1) * IBS],
                        lhsT=ktf[r0:r0 + 64, jc * 128:(jc + 1) * 128],
                        rhs=qtf[r0:r0 + 64, ib * IBS:(ib + 1) * IBS],
                        start=True, stop=True)
                ex = expp.tile([128, PACK * IBS], dt.bfloat16, tag="ex",
                               name=f"ex{g}")
                nc.scalar.activation(ex[:], st[:], EXP, scale=0.125,
                                     bias=nbias[:])
                for u, (h, ib, jc) in enumerate(pack):
                    if jc == 0:
                        ops[(h, ib)] = p2op.tile([128, IBS], dt.float32,
                                                 tag="op", name=f"opp{h}_{ib}")
                    nc.tensor.matmul(
                        ops[(h, ib)][:], lhsT=VP[jc][:, h * 128:(h + 1) * 128],
                        rhs=ex[:, u * IBS:(u + 1) * IBS],
                        start=(jc == 0), stop=(jc == NT - 1))
                    if jc == NT - 1:
                        op = ops.pop((h, ib))
                        nc.vector.tensor_copy(OPS[h][ib][:], op[0:65, :])

        # ---- P3: normalize + output projection (3-stage pipeline) ----
        with tc.tile_pool(name="p3tr", bufs=2, space="PSUM") as p3tr, \
                tc.tile_pool(name="p3tb", bufs=2, space="PSUM") as p3tb, \
                tc.tile_pool(name="p3pp", bufs=2, space="PSUM") as p3pp, \
                tc.tile_pool(name="otokp", bufs=3) as otokp, \
                tc.tile_pool(name="otnp", bufs=3) as otnp, \
                tc.tile_pool(name="linvp", bufs=3) as linvp, \
                tc.tile_pool(name="outst", bufs=2) as outst:
            otoks = {}
            otns = {}

            def tr_block(isub):
                ib, col = isub // 4, (isub % 4) * 128
                trp = p3tr.tile([128, HPC * 65], dt.float32, tag="tr",
                                name=f"trp{isub}")
                for h in range(HPC):
                    nc.tensor.transpose(trp[:, h * 65:(h + 1) * 65],
                                        OPS[h][ib][0:65, col:col + 128],
                                        id_f32[0:65, 0:65])
                trv = trp.rearrange("p (h c) -> p h c", c=65)
                linv6 = linvp.tile([128, HPC, 1], dt.float32, tag="l6",
                                   name=f"l6{isub}")
                nc.vector.reciprocal(linv6[:], trv[:, :, 64:65])
                otok = otokp.tile([128, FPC], dt.bfloat16, tag="otok",
                                  name=f"otok{isub}")
                a, b = broadcast_tensor_aps(trv[:, :, 0:64], linv6[:])
                nc.vector.tensor_mul(
                    otok.rearrange("p (h c) -> p h c", c=64), a, b)
                otoks[isub] = otok

            def tb_block(isub):
                otok = otoks.pop(isub)
                tbp = p3tb.tile([128, FPC], dt.bfloat16, tag="tb",
                                name=f"tbp{isub}")
                for f in range(FT):
                    nc.tensor.transpose(tbp[:, f * 128:(f + 1) * 128],
                                        otok[:, f * 128:(f + 1) * 128],
                                        id_bf[:])
                otn = otnp.tile([128, FPC], dt.bfloat16, tag="otn",
                                name=f"otn{isub}")
                nc.vector.tensor_copy(otn[:], tbp[:])
                otns[isub] = otn

            def proj_block(isub):
                otn = otns.pop(isub)
                pp = p3pp.tile([128, DIM], dt.float32, tag="pp",
                               name=f"pp{isub}")
                for f in range(FT):
                    nc.tensor.matmul(pp[:, 0:512], lhsT=otn[:, f * 128:(f + 1) * 128],
                                     rhs=wos[f][:, 0:512],
                                     start=(f == 0), stop=(f == FT - 1))
                    nc.tensor.matmul(pp[:, 512:768], lhsT=otn[:, f * 128:(f + 1) * 128],
                                     rhs=wos[f][:, 512:768],
                                     start=(f == 0), stop=(f == FT - 1))
                ob = outst.tile([128, DIM], dt.float32, tag="ob",
                                name=f"ob{isub}")
                nc.vector.tensor_copy(ob[:], pp[:])
                nc.sync.dma_start(out=out[isub * 128:(isub + 1) * 128, :],
                                  in_=ob[:])

            stages = [tr_block, tb_block, proj_block]
            for step in range(NT + 2):
                for depth, fn in enumerate(stages):
                    i = step - depth
                    if 0 <= i < NT:
                        fn(i)

    nc.finalize()
    return nc


def _get_nc():
    if "nc" not in _cache:
        _cache["nc"] = _build_nc()
    return _cache["nc"]


def kernel(x, Wq, Wk, Wv, Wo, bo):
    global last_exec_time_ns
    x = np.asarray(x, dtype=np.float32)
    Wq = np.asarray(Wq, dtype=np.float32)
    Wk = np.asarray(Wk, dtype=np.float32)
    Wv = np.asarray(Wv, dtype=np.float32)
    Wo = np.asarray(Wo, dtype=np.float32)
    bo = np.asarray(bo, dtype=np.float32)

    trace = bool(os.environ.get("BASS_KERNEL_TRACE"))
    if trace:
        _install_ntff_hook()
        import concourse.bass_utils as bass_utils
        bass_utils.upload_artifacts = lambda tmpdir: tmpdir

    nc = _get_nc()
    in_maps = []
    for c in range(NCORES):
        bi, hg = divmod(c, 2)
        s = slice(hg * FPC, (hg + 1) * FPC)
        in_maps.append({
            "xT": np.ascontiguousarray(x[bi].T).astype(BF16),
            "wq": np.ascontiguousarray(Wq[:, s]).astype(BF16),
            "wk": np.ascontiguousarray(Wk[:, s]).astype(BF16),
            "wv": np.ascontiguousarray(Wv[:, s]).astype(BF16),
            "wo": np.ascontiguousarray(Wo[s, :]).astype(BF16),
        })

    from concourse.bass_utils import run_bass_kernel_spmd
    res = run_bass_kernel_spmd(nc, in_maps, list(range(NCORES)), trace=trace)
    last_exec_time_ns = res.exec_time_ns

    parts = [res.results[c]["out"] for c in range(NCORES)]
    full = np.empty((B, N, DIM), np.float32)
    for bi in range(B):
        full[bi] = parts[2 * bi] + parts[2 * bi + 1] + bo[None, :]
    return full
